# revision 20
# baseline (speedup 1.0000x reference)
"""Trainium2 Bass kernel for nn_Model1_52518860096440 (dense_transformer).

Reference computation (B=4, S=4096, HID=1024, H=16, DH=64):
    qkv = query @ W_qkv.T + b_qkv            # only `query` is used
    q, k, v = split(qkv); reshape to (B,S,H,DH)
    s = einsum('bshd,bsgd->bshg', q, k) / 8 + attn_mask   # per-position head mixing
    p = softmax(s, -1)
    out = einsum('bshg,bsgd->bshd', p, v).reshape(B,S,HID)

Strategy: shard the B*S = 16384 tokens across 8 cores (2048 each).
W_qkv is replicated. Per core:
  - Phase 1: QKV projection as fp32r matmuls (full PE rate at N=512),
    lhsT = query^T tiles (host-transposed), rhs = W^T tiles (host-transposed,
    attention scale 1/8 pre-folded into the q columns of W and b).
  - Phase 2: per-position 16x16 attention via fused vector ops:
    scores: per k-head g, tensor_mul (q-part x k_g broadcast) + tensor_reduce
    softmax: mask add + exp (ACT) + strided reduce + reciprocal
    AV: scalar_tensor_tensor accumulation chains, h-split across DVE/GPSIMD.
"""

from contextlib import ExitStack

import numpy as np

B, S, HID, H = 4, 4096, 1024, 16
DH = HID // H                 # 64
NCORES = 8
T = B * S                     # 16384 tokens
TC = T // NCORES              # 2048 tokens per core
P = 128                       # partitions / tokens per tile
NT = TC // P                  # 16 token tiles per core
KT = HID // P                 # 8 contraction tiles
OC = 512                      # output-chunk for QKV matmuls
NOC = 3 * HID // OC           # 6 chunks
H_DVE = 10                    # h-heads handled on DVE (rest on GPSIMD)

_compiled = {}


def _build(phase=4, sc_gps=16, av_gps=10):
    import concourse.bass as bass
    import concourse.tile as tile
    import concourse.mybir as mybir
    from concourse import bacc

    f32 = mybir.dt.float32
    f16 = mybir.dt.float16
    f32r = mybir.dt.float32r
    Alu = mybir.AluOpType
    Act = mybir.ActivationFunctionType

    nc = bacc.Bacc("TRN2", target_bir_lowering=False, debug=False,
                   num_devices=NCORES)

    xT_d = nc.dram_tensor("xT", (HID, TC), f32r, kind="ExternalInput")
    wT_d = nc.dram_tensor("wT", (HID, 3 * HID), f32r, kind="ExternalInput")
    bias_d = nc.dram_tensor("biasbc", (P, 3 * HID), f32, kind="ExternalInput")
    mask_d = nc.dram_tensor("maskp", (TC, H * H), f32, kind="ExternalInput")
    out_d = nc.dram_tensor("out", (TC, HID), f32, kind="ExternalOutput")

    with tile.TileContext(nc) as tc, ExitStack() as ctx:
        const = ctx.enter_context(tc.tile_pool(name="const", bufs=1))
        xpool = ctx.enter_context(tc.tile_pool(name="x", bufs=3))
        qkvp = ctx.enter_context(tc.tile_pool(name="qkv", bufs=3))
        work = ctx.enter_context(tc.tile_pool(name="work", bufs=4))
        opool = ctx.enter_context(tc.tile_pool(name="o", bufs=2))
        psum = ctx.enter_context(tc.tile_pool(name="ps", bufs=3, space="PSUM"))

        # ---- resident weights / bias ----
        w_tiles = []
        wT_r = wT_d[:].rearrange("(kt kp) o -> kp kt o", kp=P)
        for kt in range(KT):
            row = []
            for oc in range(NOC):
                wt = const.tile([P, OC], f32r, tag=f"w{kt}_{oc}")
                nc.sync.dma_start(wt[:], wT_r[:, kt, oc * OC:(oc + 1) * OC])
                row.append(wt)
            w_tiles.append(row)
        bias_t = const.tile([P, 3 * HID], f32)
        nc.sync.dma_start(bias_t[:], bias_d[:])
        neg4 = const.tile([P, 1], f32, tag="neg4")
        nc.vector.memset(neg4[:], -4.0)

        xT_r = xT_d[:].rearrange("(kt kp) t -> kp kt t", kp=P)

        for tt in range(NT):
            tsl = slice(tt * P, (tt + 1) * P)

            # ---- phase 1: QKV = x @ W^T + b ----
            x_tiles = []
            for kt in range(KT):
                xk = xpool.tile([P, P], f32r, tag=f"x{kt}")
                nc.sync.dma_start(xk[:], xT_r[:, kt, tsl])
                x_tiles.append(xk)

            qkv = qkvp.tile([P, 3 * HID], f16, tag="qkv")
            for oc in range(NOC):
                acc = psum.tile([P, OC], f32, tag="acc")
                for kt in range(KT):
                    nc.tensor.matmul(acc[:], x_tiles[kt][:],
                                     w_tiles[kt][oc][:],
                                     start=(kt == 0), stop=(kt == KT - 1))
                osl = slice(oc * OC, (oc + 1) * OC)
                # psum -> sbuf with bias add (gpsimd cannot read PSUM)
                nc.vector.tensor_add(qkv[:, osl], acc[:], bias_t[:, osl])

            if phase <= 1:
                nc.sync.dma_start(out_d[tsl, :], qkv[:, 0:HID])
                continue

            qp = qkv[:, 0:HID].rearrange("p (h d) -> p h d", d=DH)

            # ---- phase 2a: scores s[t, g*16+h] = sum_d q[t,h,d] k[t,g,d] ----
            s_t = work.tile([P, H * H], f32, tag="s")
            for g in range(H):
                kg = qkv[:, HID + g * DH: HID + (g + 1) * DH]
                kg_b = kg.unsqueeze(1).broadcast_to((P, H, DH))
                tmp = work.tile([P, H, DH], f16, tag=f"tmp{g % 2}")
                mul_eng = nc.gpsimd if g < sc_gps else nc.vector
                mul_eng.tensor_mul(tmp[:], qp, kg_b)
                nc.vector.tensor_reduce(
                    s_t[:, g * H:(g + 1) * H], tmp[:],
                    axis=mybir.AxisListType.X, op=Alu.add)

            if phase <= 2:
                nc.sync.dma_start(out_d[tsl, 0:H * H], s_t[:])
                continue

            # ---- phase 2b: softmax (no max-sub; logits are O(10)) ----
            m_t = work.tile([P, H * H], f32, tag="m")
            nc.sync.dma_start(m_t[:], mask_d[tsl, :])
            sm_t = work.tile([P, H * H], f32, tag="sm")
            nc.vector.tensor_add(sm_t[:], s_t[:], m_t[:])
            e_t = work.tile([P, H * H], f16, tag="e")
            # exp(x - 4): constant shift cancels in softmax, keeps f16 finite
            nc.scalar.activation(e_t[:], sm_t[:], Act.Exp, bias=neg4[:])
            sums = work.tile([P, H], f32, tag="sums")
            nc.vector.tensor_reduce(
                sums[:], e_t[:].rearrange("p (g h) -> p h g", g=H),
                axis=mybir.AxisListType.X, op=Alu.add)
            recip = work.tile([P, H], f32, tag="recip")
            nc.vector.reciprocal(recip[:], sums[:])

            if phase <= 3:
                nc.sync.dma_start(out_d[tsl, 0:H * H], e_t[:])
                continue

            # ---- phase 2c: AV = sum_g p[t,h,g] v[t,g,:] ----
            # per h: gpsimd broadcast-mult over (g,d), DVE strided reduce over g
            vpart = qkv[:, 2 * HID:3 * HID].rearrange("p (g d) -> p g d", d=DH)
            o_t = opool.tile([P, HID], f32, tag="out")
            e3 = e_t[:].rearrange("p (g h) -> p g h", g=H)
            for h in range(H):
                # eh_b[t, g, d] = e[t, g*16+h]  (broadcast over d)
                eh_b = e3[:, :, h].unsqueeze(2).broadcast_to((P, H, DH))
                tmp = work.tile([P, H, DH], f16, tag=f"avt{h % 2}")
                mul_eng = nc.gpsimd if h < av_gps else nc.vector
                mul_eng.tensor_mul(tmp[:], vpart, eh_b)
                # reduce over g: view tmp as (p, d, g) via strides
                nc.vector.tensor_reduce(
                    o_t[:, h * DH:(h + 1) * DH],
                    tmp[:].transpose([0, 2, 1]),
                    axis=mybir.AxisListType.X, op=Alu.add)

            # ---- normalize and store ----
            r_b = recip[:].unsqueeze(2).broadcast_to((P, H, DH))
            of = opool.tile([P, HID], f32, tag="outf")
            nc.vector.tensor_mul(
                of[:].rearrange("p (h d) -> p h d", d=DH),
                o_t[:].rearrange("p (h d) -> p h d", d=DH), r_b)
            nc.sync.dma_start(out_d[tsl, :], of[:])

    nc.compile()
    return nc


def _host_prep(query, W_qkv, b_qkv, attn_mask):
    x = np.ascontiguousarray(query.reshape(T, HID), dtype=np.float32)
    xT = np.ascontiguousarray(x.T)                       # (HID, T)
    wT = np.ascontiguousarray(W_qkv.T, dtype=np.float32)  # (HID, 3*HID)
    b = np.array(b_qkv, dtype=np.float32).copy()
    scale = 1.0 / np.sqrt(DH)
    wT[:, 0:HID] *= scale
    b[0:HID] *= scale
    bias_bc = np.ascontiguousarray(np.broadcast_to(b, (P, 3 * HID)))
    # mask packed as [t, g*16+h] = attn_mask[t, h, g]
    m = np.asarray(attn_mask, dtype=np.float32).reshape(T, H, H)
    maskp = np.ascontiguousarray(m.transpose(0, 2, 1).reshape(T, H * H))
    return xT, wT, bias_bc, maskp


def kernel(query, key, value, attn_mask, W_qkv, b_qkv):
    from concourse.bass_utils import run_bass_kernel_spmd

    xT, wT, bias_bc, maskp = _host_prep(query, W_qkv, b_qkv, attn_mask)

    if "nc" not in _compiled:
        _compiled["nc"] = _build()
    nc = _compiled["nc"]

    in_maps = []
    for c in range(NCORES):
        tsl = slice(c * TC, (c + 1) * TC)
        in_maps.append({
            "xT": np.ascontiguousarray(xT[:, tsl]),
            "wT": wT,
            "biasbc": bias_bc,
            "maskp": np.ascontiguousarray(maskp[tsl, :]),
        })

    res = run_bass_kernel_spmd(nc, in_maps, core_ids=list(range(NCORES)))
    out = np.concatenate([r["out"] for r in res.results], axis=0)
    return out.reshape(B, S, HID).astype(np.float32)


if __name__ == "__main__":
    rng = np.random.default_rng(0)
    inputs = {
        "query": rng.standard_normal((B, S, HID), dtype=np.float32),
        "key": rng.standard_normal((B, S, HID), dtype=np.float32),
        "value": rng.standard_normal((B, S, HID), dtype=np.float32),
        "attn_mask": rng.standard_normal((B, S, H, H), dtype=np.float32),
        "W_qkv": (rng.standard_normal((3 * HID, HID), dtype=np.float32)
                  / np.sqrt(HID)),
        "b_qkv": rng.standard_normal((3 * HID,), dtype=np.float32) * 0.01,
    }
    out = kernel(**inputs)
    print("kernel output:", out.shape, out.dtype, np.abs(out).mean())


# revision 21
# speedup vs baseline: 1.0056x; 1.0056x over previous
"""Trainium2 Bass kernel for nn_Model1_52518860096440 (dense_transformer).

Reference computation (B=4, S=4096, HID=1024, H=16, DH=64):
    qkv = query @ W_qkv.T + b_qkv            # only `query` is used
    q, k, v = split(qkv); reshape to (B,S,H,DH)
    s = einsum('bshd,bsgd->bshg', q, k) / 8 + attn_mask   # per-position head mixing
    p = softmax(s, -1)
    out = einsum('bshg,bsgd->bshd', p, v).reshape(B,S,HID)

Strategy: shard the B*S = 16384 tokens across 8 cores (2048 each).
W_qkv is replicated. Per core:
  - Phase 1: QKV projection as fp32r matmuls (full PE rate at N=512),
    lhsT = query^T tiles (host-transposed), rhs = W^T tiles (host-transposed,
    attention scale 1/8 pre-folded into the q columns of W and b).
  - Phase 2: per-position 16x16 attention via fused vector ops:
    scores: per k-head g, tensor_mul (q-part x k_g broadcast) + tensor_reduce
    softmax: mask add + exp (ACT) + strided reduce + reciprocal
    AV: scalar_tensor_tensor accumulation chains, h-split across DVE/GPSIMD.
"""

from contextlib import ExitStack

import numpy as np

B, S, HID, H = 4, 4096, 1024, 16
DH = HID // H                 # 64
NCORES = 8
T = B * S                     # 16384 tokens
TC = T // NCORES              # 2048 tokens per core
P = 128                       # partitions / tokens per tile
NT = TC // P                  # 16 token tiles per core
KT = HID // P                 # 8 contraction tiles
OC = 512                      # output-chunk for QKV matmuls
NOC = 3 * HID // OC           # 6 chunks
H_DVE = 10                    # h-heads handled on DVE (rest on GPSIMD)

_compiled = {}


def _build(phase=4, sc_gps=14, av_gps=11):
    import concourse.bass as bass
    import concourse.tile as tile
    import concourse.mybir as mybir
    from concourse import bacc

    f32 = mybir.dt.float32
    f16 = mybir.dt.float16
    f32r = mybir.dt.float32r
    Alu = mybir.AluOpType
    Act = mybir.ActivationFunctionType

    nc = bacc.Bacc("TRN2", target_bir_lowering=False, debug=False,
                   num_devices=NCORES)

    xT_d = nc.dram_tensor("xT", (HID, TC), f32r, kind="ExternalInput")
    wT_d = nc.dram_tensor("wT", (HID, 3 * HID), f32r, kind="ExternalInput")
    bias_d = nc.dram_tensor("biasbc", (P, 3 * HID), f32, kind="ExternalInput")
    mask_d = nc.dram_tensor("maskp", (TC, H * H), f32, kind="ExternalInput")
    out_d = nc.dram_tensor("out", (TC, HID), f32, kind="ExternalOutput")

    with tile.TileContext(nc) as tc, ExitStack() as ctx:
        const = ctx.enter_context(tc.tile_pool(name="const", bufs=1))
        xpool = ctx.enter_context(tc.tile_pool(name="x", bufs=3))
        qkvp = ctx.enter_context(tc.tile_pool(name="qkv", bufs=3))
        work = ctx.enter_context(tc.tile_pool(name="work", bufs=4))
        opool = ctx.enter_context(tc.tile_pool(name="o", bufs=2))
        psum = ctx.enter_context(tc.tile_pool(name="ps", bufs=3, space="PSUM"))

        # ---- resident weights / bias ----
        w_tiles = []
        wT_r = wT_d[:].rearrange("(kt kp) o -> kp kt o", kp=P)
        for kt in range(KT):
            row = []
            for oc in range(NOC):
                wt = const.tile([P, OC], f32r, tag=f"w{kt}_{oc}")
                nc.sync.dma_start(wt[:], wT_r[:, kt, oc * OC:(oc + 1) * OC])
                row.append(wt)
            w_tiles.append(row)
        bias_t = const.tile([P, 3 * HID], f32)
        nc.sync.dma_start(bias_t[:], bias_d[:])
        neg4 = const.tile([P, 1], f32, tag="neg4")
        nc.vector.memset(neg4[:], -4.0)

        xT_r = xT_d[:].rearrange("(kt kp) t -> kp kt t", kp=P)

        for tt in range(NT):
            tsl = slice(tt * P, (tt + 1) * P)

            # ---- phase 1: QKV = x @ W^T + b ----
            x_tiles = []
            for kt in range(KT):
                xk = xpool.tile([P, P], f32r, tag=f"x{kt}")
                nc.sync.dma_start(xk[:], xT_r[:, kt, tsl])
                x_tiles.append(xk)

            qkv = qkvp.tile([P, 3 * HID], f16, tag="qkv")
            for oc in range(NOC):
                acc = psum.tile([P, OC], f32, tag="acc")
                for kt in range(KT):
                    nc.tensor.matmul(acc[:], x_tiles[kt][:],
                                     w_tiles[kt][oc][:],
                                     start=(kt == 0), stop=(kt == KT - 1))
                osl = slice(oc * OC, (oc + 1) * OC)
                # psum -> sbuf with bias add (gpsimd cannot read PSUM)
                nc.vector.tensor_add(qkv[:, osl], acc[:], bias_t[:, osl])

            if phase <= 1:
                nc.sync.dma_start(out_d[tsl, :], qkv[:, 0:HID])
                continue

            qp = qkv[:, 0:HID].rearrange("p (h d) -> p h d", d=DH)

            # ---- phase 2a: scores s[t, g*16+h] = sum_d q[t,h,d] k[t,g,d] ----
            s_t = work.tile([P, H * H], f32, tag="s")
            for g in range(H):
                kg = qkv[:, HID + g * DH: HID + (g + 1) * DH]
                kg_b = kg.unsqueeze(1).broadcast_to((P, H, DH))
                tmp = work.tile([P, H, DH], f16, tag=f"tmp{g % 2}")
                mul_eng = nc.gpsimd if g < sc_gps else nc.vector
                mul_eng.tensor_mul(tmp[:], qp, kg_b)
                nc.vector.tensor_reduce(
                    s_t[:, g * H:(g + 1) * H], tmp[:],
                    axis=mybir.AxisListType.X, op=Alu.add)

            if phase <= 2:
                nc.sync.dma_start(out_d[tsl, 0:H * H], s_t[:])
                continue

            # ---- phase 2b: softmax (no max-sub; logits are O(10)) ----
            m_t = work.tile([P, H * H], f32, tag="m")
            nc.sync.dma_start(m_t[:], mask_d[tsl, :])
            sm_t = work.tile([P, H * H], f32, tag="sm")
            nc.vector.tensor_add(sm_t[:], s_t[:], m_t[:])
            e_t = work.tile([P, H * H], f16, tag="e")
            # exp(x - 4): constant shift cancels in softmax, keeps f16 finite
            nc.scalar.activation(e_t[:], sm_t[:], Act.Exp, bias=neg4[:])
            sums = work.tile([P, H], f32, tag="sums")
            nc.vector.tensor_reduce(
                sums[:], e_t[:].rearrange("p (g h) -> p h g", g=H),
                axis=mybir.AxisListType.X, op=Alu.add)
            recip = work.tile([P, H], f32, tag="recip")
            nc.vector.reciprocal(recip[:], sums[:])

            if phase <= 3:
                nc.sync.dma_start(out_d[tsl, 0:H * H], e_t[:])
                continue

            # ---- phase 2c: AV = sum_g p[t,h,g] v[t,g,:] ----
            # per h: gpsimd broadcast-mult over (g,d), DVE strided reduce over g
            vpart = qkv[:, 2 * HID:3 * HID].rearrange("p (g d) -> p g d", d=DH)
            o_t = opool.tile([P, HID], f32, tag="out")
            e3 = e_t[:].rearrange("p (g h) -> p g h", g=H)
            for h in range(H):
                # eh_b[t, g, d] = e[t, g*16+h]  (broadcast over d)
                eh_b = e3[:, :, h].unsqueeze(2).broadcast_to((P, H, DH))
                tmp = work.tile([P, H, DH], f16, tag=f"avt{h % 2}")
                mul_eng = nc.gpsimd if h < av_gps else nc.vector
                mul_eng.tensor_mul(tmp[:], vpart, eh_b)
                # reduce over g: view tmp as (p, d, g) via strides
                nc.vector.tensor_reduce(
                    o_t[:, h * DH:(h + 1) * DH],
                    tmp[:].transpose([0, 2, 1]),
                    axis=mybir.AxisListType.X, op=Alu.add)

            # ---- normalize and store ----
            r_b = recip[:].unsqueeze(2).broadcast_to((P, H, DH))
            of = opool.tile([P, HID], f32, tag="outf")
            nc.vector.tensor_mul(
                of[:].rearrange("p (h d) -> p h d", d=DH),
                o_t[:].rearrange("p (h d) -> p h d", d=DH), r_b)
            nc.sync.dma_start(out_d[tsl, :], of[:])

    nc.compile()
    return nc


def _host_prep(query, W_qkv, b_qkv, attn_mask):
    x = np.ascontiguousarray(query.reshape(T, HID), dtype=np.float32)
    xT = np.ascontiguousarray(x.T)                       # (HID, T)
    wT = np.ascontiguousarray(W_qkv.T, dtype=np.float32)  # (HID, 3*HID)
    b = np.array(b_qkv, dtype=np.float32).copy()
    scale = 1.0 / np.sqrt(DH)
    wT[:, 0:HID] *= scale
    b[0:HID] *= scale
    bias_bc = np.ascontiguousarray(np.broadcast_to(b, (P, 3 * HID)))
    # mask packed as [t, g*16+h] = attn_mask[t, h, g]
    m = np.asarray(attn_mask, dtype=np.float32).reshape(T, H, H)
    maskp = np.ascontiguousarray(m.transpose(0, 2, 1).reshape(T, H * H))
    return xT, wT, bias_bc, maskp


def kernel(query, key, value, attn_mask, W_qkv, b_qkv):
    from concourse.bass_utils import run_bass_kernel_spmd

    xT, wT, bias_bc, maskp = _host_prep(query, W_qkv, b_qkv, attn_mask)

    if "nc" not in _compiled:
        _compiled["nc"] = _build()
    nc = _compiled["nc"]

    in_maps = []
    for c in range(NCORES):
        tsl = slice(c * TC, (c + 1) * TC)
        in_maps.append({
            "xT": np.ascontiguousarray(xT[:, tsl]),
            "wT": wT,
            "biasbc": bias_bc,
            "maskp": np.ascontiguousarray(maskp[tsl, :]),
        })

    res = run_bass_kernel_spmd(nc, in_maps, core_ids=list(range(NCORES)))
    out = np.concatenate([r["out"] for r in res.results], axis=0)
    return out.reshape(B, S, HID).astype(np.float32)


if __name__ == "__main__":
    rng = np.random.default_rng(0)
    inputs = {
        "query": rng.standard_normal((B, S, HID), dtype=np.float32),
        "key": rng.standard_normal((B, S, HID), dtype=np.float32),
        "value": rng.standard_normal((B, S, HID), dtype=np.float32),
        "attn_mask": rng.standard_normal((B, S, H, H), dtype=np.float32),
        "W_qkv": (rng.standard_normal((3 * HID, HID), dtype=np.float32)
                  / np.sqrt(HID)),
        "b_qkv": rng.standard_normal((3 * HID,), dtype=np.float32) * 0.01,
    }
    out = kernel(**inputs)
    print("kernel output:", out.shape, out.dtype, np.abs(out).mean())


# revision 23
# speedup vs baseline: 1.0170x; 1.0114x over previous
"""Trainium2 Bass kernel for nn_Model1_52518860096440 (dense_transformer).

Reference computation (B=4, S=4096, HID=1024, H=16, DH=64):
    qkv = query @ W_qkv.T + b_qkv            # only `query` is used
    q, k, v = split(qkv); reshape to (B,S,H,DH)
    s = einsum('bshd,bsgd->bshg', q, k) / 8 + attn_mask   # per-position head mixing
    p = softmax(s, -1)
    out = einsum('bshg,bsgd->bshd', p, v).reshape(B,S,HID)

Strategy: shard the B*S = 16384 tokens across 8 cores (2048 each).
W_qkv is replicated. Per core:
  - Phase 1: QKV projection as fp32r matmuls (full PE rate at N=512),
    lhsT = query^T tiles (host-transposed), rhs = W^T tiles (host-transposed,
    attention scale 1/8 pre-folded into the q columns of W and b).
  - Phase 2: per-position 16x16 attention via fused vector ops:
    scores: per k-head g, tensor_mul (q-part x k_g broadcast) + tensor_reduce
    softmax: mask add + exp (ACT) + strided reduce + reciprocal
    AV: scalar_tensor_tensor accumulation chains, h-split across DVE/GPSIMD.
"""

from contextlib import ExitStack

import numpy as np

B, S, HID, H = 4, 4096, 1024, 16
DH = HID // H                 # 64
NCORES = 8
T = B * S                     # 16384 tokens
TC = T // NCORES              # 2048 tokens per core
P = 128                       # partitions / tokens per tile
NT = TC // P                  # 16 token tiles per core
KT = HID // P                 # 8 contraction tiles
OC = 512                      # output-chunk for QKV matmuls
NOC = 3 * HID // OC           # 6 chunks
H_DVE = 10                    # h-heads handled on DVE (rest on GPSIMD)

_compiled = {}


def _build(phase=4, sc_gps=14, av_gps=12):
    import concourse.bass as bass
    import concourse.tile as tile
    import concourse.mybir as mybir
    from concourse import bacc

    f32 = mybir.dt.float32
    f16 = mybir.dt.float16
    f32r = mybir.dt.float32r
    Alu = mybir.AluOpType
    Act = mybir.ActivationFunctionType

    nc = bacc.Bacc("TRN2", target_bir_lowering=False, debug=False,
                   num_devices=NCORES)

    xT_d = nc.dram_tensor("xT", (HID, TC), f32r, kind="ExternalInput")
    wT_d = nc.dram_tensor("wT", (HID, 3 * HID), f32r, kind="ExternalInput")
    bias_d = nc.dram_tensor("biasbc", (P, 3 * HID), f32, kind="ExternalInput")
    mask_d = nc.dram_tensor("maskp", (TC, H * H), f32, kind="ExternalInput")
    out_d = nc.dram_tensor("out", (TC, HID), f32, kind="ExternalOutput")

    with tile.TileContext(nc) as tc, ExitStack() as ctx:
        const = ctx.enter_context(tc.tile_pool(name="const", bufs=1))
        xpool = ctx.enter_context(tc.tile_pool(name="x", bufs=3))
        qkvp = ctx.enter_context(tc.tile_pool(name="qkv", bufs=3))
        work = ctx.enter_context(tc.tile_pool(name="work", bufs=4))
        opool = ctx.enter_context(tc.tile_pool(name="o", bufs=2))
        psum = ctx.enter_context(tc.tile_pool(name="ps", bufs=3, space="PSUM"))

        # ---- resident weights / bias ----
        w_tiles = []
        wT_r = wT_d[:].rearrange("(kt kp) o -> kp kt o", kp=P)
        for kt in range(KT):
            row = []
            for oc in range(NOC):
                wt = const.tile([P, OC], f32r, tag=f"w{kt}_{oc}")
                nc.sync.dma_start(wt[:], wT_r[:, kt, oc * OC:(oc + 1) * OC])
                row.append(wt)
            w_tiles.append(row)
        bias_t = const.tile([P, 3 * HID], f32)
        nc.sync.dma_start(bias_t[:], bias_d[:])
        neg4 = const.tile([P, 1], f32, tag="neg4")
        nc.vector.memset(neg4[:], -4.0)
        ones_r = const.tile([1, P], f32, tag="ones_r")
        nc.vector.memset(ones_r[:], 1.0)

        xT_r = xT_d[:].rearrange("(kt kp) t -> kp kt t", kp=P)

        for tt in range(NT):
            tsl = slice(tt * P, (tt + 1) * P)

            # ---- phase 1: QKV = x @ W^T + b ----
            x_tiles = []
            for kt in range(KT):
                xk = xpool.tile([P, P], f32r, tag=f"x{kt}")
                nc.sync.dma_start(xk[:], xT_r[:, kt, tsl])
                x_tiles.append(xk)

            qkv = qkvp.tile([P, 3 * HID], f16, tag="qkv")
            for oc in range(NOC):
                acc = psum.tile([P, OC], f32, tag="acc")
                osl = slice(oc * OC, (oc + 1) * OC)
                for kt in range(KT):
                    nc.tensor.matmul(acc[:], x_tiles[kt][:],
                                     w_tiles[kt][oc][:],
                                     start=(kt == 0), stop=False)
                # bias as a K=1 ones-row matmul accumulated into PSUM
                nc.tensor.matmul(acc[:], ones_r[:], bias_t[0:1, osl],
                                 start=False, stop=True)
                # psum -> sbuf copy on ACT (frees DVE)
                nc.scalar.copy(qkv[:, osl], acc[:])

            if phase <= 1:
                nc.sync.dma_start(out_d[tsl, :], qkv[:, 0:HID])
                continue

            qp = qkv[:, 0:HID].rearrange("p (h d) -> p h d", d=DH)

            # ---- phase 2a: scores s[t, g*16+h] = sum_d q[t,h,d] k[t,g,d] ----
            s_t = work.tile([P, H * H], f32, tag="s")
            for g in range(H):
                kg = qkv[:, HID + g * DH: HID + (g + 1) * DH]
                kg_b = kg.unsqueeze(1).broadcast_to((P, H, DH))
                tmp = work.tile([P, H, DH], f16, tag=f"tmp{g % 2}")
                mul_eng = nc.gpsimd if g < sc_gps else nc.vector
                mul_eng.tensor_mul(tmp[:], qp, kg_b)
                nc.vector.tensor_reduce(
                    s_t[:, g * H:(g + 1) * H], tmp[:],
                    axis=mybir.AxisListType.X, op=Alu.add)

            if phase <= 2:
                nc.sync.dma_start(out_d[tsl, 0:H * H], s_t[:])
                continue

            # ---- phase 2b: softmax (no max-sub; logits are O(10)) ----
            m_t = work.tile([P, H * H], f32, tag="m")
            nc.sync.dma_start(m_t[:], mask_d[tsl, :])
            sm_t = work.tile([P, H * H], f32, tag="sm")
            nc.vector.tensor_add(sm_t[:], s_t[:], m_t[:])
            e_t = work.tile([P, H * H], f16, tag="e")
            # exp(x - 4): constant shift cancels in softmax, keeps f16 finite
            nc.scalar.activation(e_t[:], sm_t[:], Act.Exp, bias=neg4[:])
            sums = work.tile([P, H], f32, tag="sums")
            nc.vector.tensor_reduce(
                sums[:], e_t[:].rearrange("p (g h) -> p h g", g=H),
                axis=mybir.AxisListType.X, op=Alu.add)
            recip = work.tile([P, H], f32, tag="recip")
            nc.vector.reciprocal(recip[:], sums[:])

            if phase <= 3:
                nc.sync.dma_start(out_d[tsl, 0:H * H], e_t[:])
                continue

            # ---- phase 2c: AV = sum_g p[t,h,g] v[t,g,:] ----
            # per h: gpsimd broadcast-mult over (g,d), DVE strided reduce over g
            vpart = qkv[:, 2 * HID:3 * HID].rearrange("p (g d) -> p g d", d=DH)
            o_t = opool.tile([P, HID], f32, tag="out")
            e3 = e_t[:].rearrange("p (g h) -> p g h", g=H)
            for h in range(H):
                # eh_b[t, g, d] = e[t, g*16+h]  (broadcast over d)
                eh_b = e3[:, :, h].unsqueeze(2).broadcast_to((P, H, DH))
                tmp = work.tile([P, H, DH], f16, tag=f"avt{h % 2}")
                mul_eng = nc.gpsimd if h < av_gps else nc.vector
                mul_eng.tensor_mul(tmp[:], vpart, eh_b)
                # reduce over g: view tmp as (p, d, g) via strides
                nc.vector.tensor_reduce(
                    o_t[:, h * DH:(h + 1) * DH],
                    tmp[:].transpose([0, 2, 1]),
                    axis=mybir.AxisListType.X, op=Alu.add)

            # ---- normalize and store ----
            r_b = recip[:].unsqueeze(2).broadcast_to((P, H, DH))
            of = opool.tile([P, HID], f32, tag="outf")
            nc.vector.tensor_mul(
                of[:].rearrange("p (h d) -> p h d", d=DH),
                o_t[:].rearrange("p (h d) -> p h d", d=DH), r_b)
            nc.sync.dma_start(out_d[tsl, :], of[:])

    nc.compile()
    return nc


def _host_prep(query, W_qkv, b_qkv, attn_mask):
    x = np.ascontiguousarray(query.reshape(T, HID), dtype=np.float32)
    xT = np.ascontiguousarray(x.T)                       # (HID, T)
    wT = np.ascontiguousarray(W_qkv.T, dtype=np.float32)  # (HID, 3*HID)
    b = np.array(b_qkv, dtype=np.float32).copy()
    scale = 1.0 / np.sqrt(DH)
    wT[:, 0:HID] *= scale
    b[0:HID] *= scale
    bias_bc = np.ascontiguousarray(np.broadcast_to(b, (P, 3 * HID)))
    # mask packed as [t, g*16+h] = attn_mask[t, h, g]
    m = np.asarray(attn_mask, dtype=np.float32).reshape(T, H, H)
    maskp = np.ascontiguousarray(m.transpose(0, 2, 1).reshape(T, H * H))
    return xT, wT, bias_bc, maskp


def kernel(query, key, value, attn_mask, W_qkv, b_qkv):
    from concourse.bass_utils import run_bass_kernel_spmd

    xT, wT, bias_bc, maskp = _host_prep(query, W_qkv, b_qkv, attn_mask)

    if "nc" not in _compiled:
        _compiled["nc"] = _build()
    nc = _compiled["nc"]

    in_maps = []
    for c in range(NCORES):
        tsl = slice(c * TC, (c + 1) * TC)
        in_maps.append({
            "xT": np.ascontiguousarray(xT[:, tsl]),
            "wT": wT,
            "biasbc": bias_bc,
            "maskp": np.ascontiguousarray(maskp[tsl, :]),
        })

    res = run_bass_kernel_spmd(nc, in_maps, core_ids=list(range(NCORES)))
    out = np.concatenate([r["out"] for r in res.results], axis=0)
    return out.reshape(B, S, HID).astype(np.float32)


if __name__ == "__main__":
    rng = np.random.default_rng(0)
    inputs = {
        "query": rng.standard_normal((B, S, HID), dtype=np.float32),
        "key": rng.standard_normal((B, S, HID), dtype=np.float32),
        "value": rng.standard_normal((B, S, HID), dtype=np.float32),
        "attn_mask": rng.standard_normal((B, S, H, H), dtype=np.float32),
        "W_qkv": (rng.standard_normal((3 * HID, HID), dtype=np.float32)
                  / np.sqrt(HID)),
        "b_qkv": rng.standard_normal((3 * HID,), dtype=np.float32) * 0.01,
    }
    out = kernel(**inputs)
    print("kernel output:", out.shape, out.dtype, np.abs(out).mean())
